# revision 23
# baseline (speedup 1.0000x reference)
"""Trainium2 Bass kernel: per-channel 256-bin normalized histogram.

Input: full inputs [64, 512, 512, 3] float32 in [0, 1).
Output: [256, 3] float32 - per-channel histogram normalized to sum 1.

Strategy (8 NeuronCores, data-parallel over the batch dim):
  Each core gets 8 batches = 2,097,152 elements per channel, laid out
  channel-planar [128, 3*16384] fp32 (host-side relayout).

  Per core, per 1024-column chunk (one channel per chunk):
    1. Prep (ScalarE+VectorE): idx = floor(256x) via an ACT affine plus
       a DVE magic-add round (tie-fudged by -0.5 ulp); c16 = 129 +
       floor(idx/16) via an ACT affine whose bf16 output cast rounds on
       the ulp-1 grid of [128,256); fine = idx + (2064 - 16*c16).
    2. Planes: coarse side as KSGN +-1 ge-staircases (ScalarE Sign) plus
       16-KSGN one-hot planes (VectorE is_equal, 4x bf16), interleaved
       [p, batch, bin, 8] so each 8-column batch is a contiguous
       [128, 128] stationary block; fine side as 16 one-hot planes,
       flat plane-major (the moving AP may keep two free dims).
    3. Joint counts (TensorE): per batch, one LDWEIGHTS + one FD=128
       matmul: psum[a*8+r, f*8+s] += sum_p A_a(col r)[p] * F_f(col s)[p].
       Diagonal r == s cells hold the joint (coarse, fine) products;
       off-diagonal cells are garbage that accumulates harmlessly.
  One [128,128] fp32 PSUM tile per channel accumulates all 2048 batches
  (counts < 2^24, so fp32 accumulation is exact).

  Host: extracts the r == s cells, undoes the +-1 staircase algebra
  (Jge rows -> first differences), all-reduces the 8 cores' counts in
  fp64 (exact integers), applies the per-channel fp32 normalize.
"""

import os

import numpy as np

import concourse.bacc as bacc
import concourse.mybir as mybir
from concourse.bass_utils import run_bass_kernel_spmd
from concourse.tile import TileContext

# Problem constants (hardcoded per contract)
B, H, W, C = 64, 512, 512, 3
NBINS = 256
NCORES = 8
P = 128

EPC = (B // NCORES) * H * W * C       # 6,291,456 elements per core
NCH = EPC // C                        # 2,097,152 per channel per core
FCH = NCH // P                        # 16,384 columns per channel
CHUNK = 1024                          # columns per chunk
NCHUNK_CH = FCH // CHUNK              # 16 chunks per channel
GB = 8                                # data columns per matmul batch
NBATCH = CHUNK // GB                  # 128 batches per chunk
KSGN = 7                              # A-bins 0..KSGN-1 via ScalarE Sign

# Magic-round constant with +512 margin so the add always lands in the
# [2^23, 2^24) ulp-1 zone even for slightly negative inputs.
# idx = RNE((256*x - 0.5) + K1) - K1  ==  floor(256*x) up to RNE ties
# at exact multiples of 1/256 (~1e-5 of elements, half of them off by 1).
K1 = float(np.float32(2.0 ** 23 + 512))
AL = mybir.AluOpType

_CACHE: dict = {}


def _build_module():
    nc = bacc.Bacc("TRN2", target_bir_lowering=False, debug=False,
                   num_devices=NCORES)

    x_ext = nc.declare_dram_parameter("x", [P, C * FCH], mybir.dt.float32,
                                      isOutput=False)
    out_ext = nc.declare_dram_parameter("joint", [P, C * P],
                                        mybir.dt.float32, isOutput=True)

    with TileContext(nc) as tc:
        with (tc.tile_pool(name="persist", bufs=1) as pp,
              tc.tile_pool(name="chunk", bufs=2) as cp,
              tc.tile_pool(name="planes", bufs=2) as plp,
              tc.tile_pool(name="psum", bufs=1, space="PSUM") as psp):
            ps = [psp.tile([P, P], mybir.dt.float32, tag=f"ps{c}",
                           name=f"ps{c}") for c in range(C)]
            res = pp.tile([P, C * P], mybir.dt.float32, tag="res")
            bias_y = pp.tile([P, 1], mybir.dt.float32, tag="bias_y")
            bias_c = pp.tile([P, 1], mybir.dt.float32, tag="bias_c")
            nc.gpsimd.memset(bias_y[:], -0.5)
            nc.gpsimd.memset(bias_c[:], 129.0 - 0.46875)
            bias_t = pp.tile([P, 1], mybir.dt.float32, tag="bias_t")
            nc.gpsimd.memset(bias_t[:], 2064.0)
            bias_s = []
            for a in range(KSGN):
                bs = pp.tile([P, 1], mybir.dt.float32, tag=f"bs{a}",
                             name=f"bs{a}")
                nc.gpsimd.memset(bs[:], 0.5 - 16.0 * a)
                bias_s.append(bs)

            for ci in range(C):
                for k in range(NCHUNK_CH):
                    off = ci * FCH + k * CHUNK
                    xb = cp.tile([P, CHUNK], mybir.dt.float32, tag="xb")
                    af = cp.tile([P, CHUNK], mybir.dt.float32, tag="af")
                    idx = cp.tile([P, CHUNK], mybir.dt.bfloat16, tag="idx")
                    crs16 = cp.tile([P, CHUNK], mybir.dt.bfloat16, tag="c16")
                    crs = cp.tile([P, CHUNK], mybir.dt.bfloat16, tag="crs")
                    fin = cp.tile([P, CHUNK], mybir.dt.bfloat16, tag="fin")
                    apl = plp.tile([P, NBATCH, 16, GB], mybir.dt.bfloat16,
                                   tag="apl")
                    fpl = plp.tile([P, 16, CHUNK], mybir.dt.bfloat16,
                                   tag="fpl")

                    nc.sync.dma_start(out=xb[:],
                                      in_=x_ext.ap()[:, off:off + CHUNK])
                    # idx = RNE((256x - .5) + K1) - K1   (floor, tie-fudged)
                    nc.scalar.activation(
                        af[:], xb[:], mybir.ActivationFunctionType.Identity,
                        bias=bias_y[:], scale=256.0)
                    nc.vector.tensor_scalar(
                        idx[:], af[:], K1, -K1, AL.add, AL.add)
                    # c16 = 129 + floor(idx/16): the bf16 output cast rounds
                    # on the ulp-1 grid of [128, 256), doing the floor.
                    nc.scalar.activation(
                        crs16[:], idx[:],
                        mybir.ActivationFunctionType.Identity,
                        bias=bias_c[:], scale=0.0625)
                    # fine = idx + (2064 - 16*c16)  (t2 in [-240, 0], exact
                    # in bf16); A-planes compare c16 against 129+a directly.
                    nc.scalar.activation(
                        crs[:], crs16[:],
                        mybir.ActivationFunctionType.Identity,
                        bias=bias_t[:], scale=-16.0)
                    nc.vector.tensor_tensor(
                        fin[:], crs[:], idx[:], AL.add)

                    # A-planes interleaved [p, batch, bin, 8] (stationary
                    # AP must coalesce to one free dim): bins 0..KSGN-1 as
                    # +-1 ge-staircases on ScalarE (Sign(idx - 16a + .5));
                    # bins KSGN..15 as 0/1 one-hot on VectorE.
                    # F-planes flat plane-major [p, bin, col], 0/1 one-hot.
                    for a in range(KSGN):
                        nc.scalar.activation(
                            apl[:, :, a, :], idx[:],
                            mybir.ActivationFunctionType.Sign,
                            bias=bias_s[a][:], scale=1.0)
                    for b in range(KSGN, 16):
                        nc.vector.tensor_scalar(
                            apl[:, :, b, :], crs16[:], float(129 + b), None,
                            AL.is_equal)
                    for b in range(16):
                        nc.vector.tensor_scalar(
                            fpl[:, b, :], fin[:], float(b), None,
                            AL.is_equal)

                    for bt in range(NBATCH):
                        nc.tensor.matmul(
                            ps[ci][:], apl[:, bt],
                            fpl[:, :, bt * GB:(bt + 1) * GB],
                            start=(k == 0 and bt == 0),
                            stop=(k == NCHUNK_CH - 1 and bt == NBATCH - 1))

                nc.vector.tensor_copy(res[:, ci * P:(ci + 1) * P],
                                      ps[ci][:])

            nc.sync.dma_start(out=out_ext.ap(), in_=res[:])

    nc.finalize()
    return nc


def _get_module():
    if "nc" not in _CACHE:
        _CACHE["nc"] = _build_module()
    return _CACHE["nc"]


def _shard_host(x: np.ndarray) -> list[np.ndarray]:
    """[B,H,W,C] fp32 -> per-core channel-planar [P, C*FCH] arrays."""
    xs = x.reshape(NCORES, EPC // C, C)
    shards = []
    for i in range(NCORES):
        # [NCH, C] -> [C, NCH] -> [C, P, FCH] -> [P, C, FCH]
        sc = np.ascontiguousarray(
            xs[i].T.reshape(C, P, FCH).transpose(1, 0, 2)).reshape(
                P, C * FCH)
        shards.append(sc)
    return shards


def _decode_counts(results) -> np.ndarray:
    """Extract joint counts from [128, C*128] psum images -> [C, 256].

    A-rows 0..KSGN-1 hold +-1 ge-staircase products: out[a,f] =
    2*Jge[a,f] - C_f (a >= 1), out[0,f] = C_f (all-ones staircase).
    A-rows KSGN..15 hold plain one-hot joint counts. Jge[KSGN,f] is
    reconstructed from the one-hot rows; counts = first differences.
    """
    out = np.zeros((C, 16, 16), dtype=np.float64)
    r_idx = np.arange(GB)
    for r in results:
        jall = r["joint"].astype(np.float64).reshape(P, C, P)
        for ci in range(C):
            j = jall[:, ci, :]  # [128, 128]
            for a in range(16):
                for f in range(16):
                    out[ci, a, f] += j[a * GB + r_idx,
                                       f * GB + r_idx].sum()
    counts = np.zeros((C, 16, 16), dtype=np.float64)
    for ci in range(C):
        o = out[ci]
        cf = o[0]                      # fine marginals C_f
        jge = np.zeros((KSGN + 1, 16))
        jge[0] = cf
        for a in range(1, KSGN):
            jge[a] = (o[a] + cf) / 2.0
        jge[KSGN] = o[KSGN:].sum(axis=0)
        counts[ci, :KSGN] = jge[:KSGN] - jge[1:]
        counts[ci, KSGN:] = o[KSGN:]
    return counts.reshape(C, NBINS)


def run(x: np.ndarray, trace: bool = False):
    nc = _get_module()

    x = np.ascontiguousarray(x, dtype=np.float32)
    assert x.shape == (B, H, W, C)
    shards = _shard_host(x)
    in_maps = [{"x": shards[i]} for i in range(NCORES)]

    res = run_bass_kernel_spmd(nc, in_maps, list(range(NCORES)), trace=trace)

    counts = _decode_counts(res.results)
    assert counts.sum() == float(B * H * W * C), counts.sum()
    # Normalization exactly as the reference: fp32 divide, then transpose.
    counts32 = counts.astype(np.float32)
    sums = counts32.sum(axis=1, keepdims=True, dtype=np.float32)
    hist = counts32 / sums
    return np.ascontiguousarray(hist.T), res


def kernel(**inputs) -> np.ndarray:
    out, _ = run(inputs["inputs"],
                 trace=bool(os.environ.get("KERNEL_TRACE")))
    return out


# revision 24
# speedup vs baseline: 1.0063x; 1.0063x over previous
"""Trainium2 Bass kernel: per-channel 256-bin normalized histogram.

Input: full inputs [64, 512, 512, 3] float32 in [0, 1).
Output: [256, 3] float32 - per-channel histogram normalized to sum 1.

Strategy (8 NeuronCores, data-parallel over the batch dim):
  Each core gets 8 batches = 2,097,152 elements per channel, laid out
  channel-planar [128, 3*16384] fp32 (host-side relayout).

  Per core, per 1024-column chunk (one channel per chunk):
    1. Prep (ScalarE+VectorE): idx = floor(256x) via an ACT affine plus
       a DVE magic-add round (tie-fudged by -0.5 ulp); c16 = 129 +
       floor(idx/16) via an ACT affine whose bf16 output cast rounds on
       the ulp-1 grid of [128,256); fine = idx + (2064 - 16*c16).
    2. Planes: coarse side as KSGN +-1 ge-staircases (ScalarE Sign) plus
       16-KSGN one-hot planes (VectorE is_equal, 4x bf16), interleaved
       [p, batch, bin, 8] so each 8-column batch is a contiguous
       [128, 128] stationary block; fine side as 16 one-hot planes,
       flat plane-major (the moving AP may keep two free dims).
    3. Joint counts (TensorE): per batch, one LDWEIGHTS + one FD=128
       matmul: psum[a*8+r, f*8+s] += sum_p A_a(col r)[p] * F_f(col s)[p].
       Diagonal r == s cells hold the joint (coarse, fine) products;
       off-diagonal cells are garbage that accumulates harmlessly.
  One [128,128] fp32 PSUM tile per channel accumulates all 2048 batches
  (counts < 2^24, so fp32 accumulation is exact).

  Host: extracts the r == s cells, undoes the +-1 staircase algebra
  (Jge rows -> first differences), all-reduces the 8 cores' counts in
  fp64 (exact integers), applies the per-channel fp32 normalize.
"""

import os

import numpy as np

import concourse.bacc as bacc
import concourse.mybir as mybir
from concourse.bass_utils import run_bass_kernel_spmd
from concourse.tile import TileContext

# Problem constants (hardcoded per contract)
B, H, W, C = 64, 512, 512, 3
NBINS = 256
NCORES = 8
P = 128

EPC = (B // NCORES) * H * W * C       # 6,291,456 elements per core
NCH = EPC // C                        # 2,097,152 per channel per core
FCH = NCH // P                        # 16,384 columns per channel
CHUNK = 1024                          # columns per chunk
NCHUNK_CH = FCH // CHUNK              # 16 chunks per channel
GB = 8                                # data columns per matmul batch
NBATCH = CHUNK // GB                  # 128 batches per chunk
KSGN = 7                              # A-bins 0..KSGN-1 via ScalarE Sign

# Magic-round constant with +512 margin so the add always lands in the
# [2^23, 2^24) ulp-1 zone even for slightly negative inputs.
# idx = RNE((256*x - 0.5) + K1) - K1  ==  floor(256*x) up to RNE ties
# at exact multiples of 1/256 (~1e-5 of elements, half of them off by 1).
K1 = float(np.float32(2.0 ** 23 + 512))
AL = mybir.AluOpType

_CACHE: dict = {}


def _build_module():
    nc = bacc.Bacc("TRN2", target_bir_lowering=False, debug=False,
                   num_devices=NCORES)

    x_ext = nc.declare_dram_parameter("x", [P, C * FCH], mybir.dt.float32,
                                      isOutput=False)
    out_ext = nc.declare_dram_parameter("joint", [P, C * P],
                                        mybir.dt.float32, isOutput=True)

    with TileContext(nc) as tc:
        with (tc.tile_pool(name="persist", bufs=1) as pp,
              tc.tile_pool(name="chunk", bufs=2) as cp,
              tc.tile_pool(name="planes", bufs=2) as plp,
              tc.tile_pool(name="psum", bufs=1, space="PSUM") as psp):
            ps = [psp.tile([P, P], mybir.dt.float32, tag=f"ps{c}",
                           name=f"ps{c}") for c in range(C)]
            res = pp.tile([P, C * P], mybir.dt.float32, tag="res")
            bias_y = pp.tile([P, 1], mybir.dt.float32, tag="bias_y")
            bias_c = pp.tile([P, 1], mybir.dt.float32, tag="bias_c")
            nc.gpsimd.memset(bias_y[:], -0.5)
            nc.gpsimd.memset(bias_c[:], 129.0 - 0.46875)
            bias_t = pp.tile([P, 1], mybir.dt.float32, tag="bias_t")
            nc.gpsimd.memset(bias_t[:], 2064.0)
            bias_s = []
            for a in range(KSGN):
                bs = pp.tile([P, 1], mybir.dt.float32, tag=f"bs{a}",
                             name=f"bs{a}")
                nc.gpsimd.memset(bs[:], 0.5 - 16.0 * a)
                bias_s.append(bs)

            for ci in range(C):
                for k in range(NCHUNK_CH):
                    off = ci * FCH + k * CHUNK
                    xb = cp.tile([P, CHUNK], mybir.dt.float32, tag="xb")
                    af = cp.tile([P, CHUNK], mybir.dt.float32, tag="af")
                    idx = cp.tile([P, CHUNK], mybir.dt.bfloat16, tag="idx")
                    crs16 = cp.tile([P, CHUNK], mybir.dt.bfloat16, tag="c16")
                    crs = cp.tile([P, CHUNK], mybir.dt.bfloat16, tag="crs")
                    fin = cp.tile([P, CHUNK], mybir.dt.bfloat16, tag="fin")
                    apl = plp.tile([P, NBATCH, 16, GB], mybir.dt.bfloat16,
                                   tag="apl")
                    fpl = plp.tile([P, 16, CHUNK], mybir.dt.bfloat16,
                                   tag="fpl")

                    nc.sync.dma_start(out=xb[:],
                                      in_=x_ext.ap()[:, off:off + CHUNK])
                    # idx = RNE((256x - .5) + K1) - K1   (floor, tie-fudged)
                    nc.scalar.activation(
                        af[:], xb[:], mybir.ActivationFunctionType.Identity,
                        bias=bias_y[:], scale=256.0)
                    nc.vector.tensor_scalar(
                        idx[:], af[:], K1, -K1, AL.add, AL.add)
                    # c16 = 129 + floor(idx/16): the bf16 output cast rounds
                    # on the ulp-1 grid of [128, 256), doing the floor.
                    nc.scalar.activation(
                        crs16[:], idx[:],
                        mybir.ActivationFunctionType.Identity,
                        bias=bias_c[:], scale=0.0625)
                    # fine = idx + (2064 - 16*c16)  (t2 in [-240, 0], exact
                    # in bf16); A-planes compare c16 against 129+a directly.
                    nc.vector.tensor_scalar(
                        crs[:], crs16[:], -16.0, 2064.0, AL.mult, AL.add)
                    nc.vector.tensor_tensor(
                        fin[:], crs[:], idx[:], AL.add)

                    # A-planes interleaved [p, batch, bin, 8] (stationary
                    # AP must coalesce to one free dim): bins 0..KSGN-1 as
                    # +-1 ge-staircases on ScalarE (Sign(idx - 16a + .5));
                    # bins KSGN..15 as 0/1 one-hot on VectorE.
                    # F-planes flat plane-major [p, bin, col], 0/1 one-hot.
                    for a in range(KSGN):
                        nc.scalar.activation(
                            apl[:, :, a, :], idx[:],
                            mybir.ActivationFunctionType.Sign,
                            bias=bias_s[a][:], scale=1.0)
                    for b in range(KSGN, 16):
                        nc.vector.tensor_scalar(
                            apl[:, :, b, :], crs16[:], float(129 + b), None,
                            AL.is_equal)
                    for b in range(16):
                        nc.vector.tensor_scalar(
                            fpl[:, b, :], fin[:], float(b), None,
                            AL.is_equal)

                    for bt in range(NBATCH):
                        nc.tensor.matmul(
                            ps[ci][:], apl[:, bt],
                            fpl[:, :, bt * GB:(bt + 1) * GB],
                            start=(k == 0 and bt == 0),
                            stop=(k == NCHUNK_CH - 1 and bt == NBATCH - 1))

                nc.vector.tensor_copy(res[:, ci * P:(ci + 1) * P],
                                      ps[ci][:])

            nc.sync.dma_start(out=out_ext.ap(), in_=res[:])

    nc.finalize()
    return nc


def _get_module():
    if "nc" not in _CACHE:
        _CACHE["nc"] = _build_module()
    return _CACHE["nc"]


def _shard_host(x: np.ndarray) -> list[np.ndarray]:
    """[B,H,W,C] fp32 -> per-core channel-planar [P, C*FCH] arrays."""
    xs = x.reshape(NCORES, EPC // C, C)
    shards = []
    for i in range(NCORES):
        # [NCH, C] -> [C, NCH] -> [C, P, FCH] -> [P, C, FCH]
        sc = np.ascontiguousarray(
            xs[i].T.reshape(C, P, FCH).transpose(1, 0, 2)).reshape(
                P, C * FCH)
        shards.append(sc)
    return shards


def _decode_counts(results) -> np.ndarray:
    """Extract joint counts from [128, C*128] psum images -> [C, 256].

    A-rows 0..KSGN-1 hold +-1 ge-staircase products: out[a,f] =
    2*Jge[a,f] - C_f (a >= 1), out[0,f] = C_f (all-ones staircase).
    A-rows KSGN..15 hold plain one-hot joint counts. Jge[KSGN,f] is
    reconstructed from the one-hot rows; counts = first differences.
    """
    out = np.zeros((C, 16, 16), dtype=np.float64)
    r_idx = np.arange(GB)
    for r in results:
        jall = r["joint"].astype(np.float64).reshape(P, C, P)
        for ci in range(C):
            j = jall[:, ci, :]  # [128, 128]
            for a in range(16):
                for f in range(16):
                    out[ci, a, f] += j[a * GB + r_idx,
                                       f * GB + r_idx].sum()
    counts = np.zeros((C, 16, 16), dtype=np.float64)
    for ci in range(C):
        o = out[ci]
        cf = o[0]                      # fine marginals C_f
        jge = np.zeros((KSGN + 1, 16))
        jge[0] = cf
        for a in range(1, KSGN):
            jge[a] = (o[a] + cf) / 2.0
        jge[KSGN] = o[KSGN:].sum(axis=0)
        counts[ci, :KSGN] = jge[:KSGN] - jge[1:]
        counts[ci, KSGN:] = o[KSGN:]
    return counts.reshape(C, NBINS)


def run(x: np.ndarray, trace: bool = False):
    nc = _get_module()

    x = np.ascontiguousarray(x, dtype=np.float32)
    assert x.shape == (B, H, W, C)
    shards = _shard_host(x)
    in_maps = [{"x": shards[i]} for i in range(NCORES)]

    res = run_bass_kernel_spmd(nc, in_maps, list(range(NCORES)), trace=trace)

    counts = _decode_counts(res.results)
    assert counts.sum() == float(B * H * W * C), counts.sum()
    # Normalization exactly as the reference: fp32 divide, then transpose.
    counts32 = counts.astype(np.float32)
    sums = counts32.sum(axis=1, keepdims=True, dtype=np.float32)
    hist = counts32 / sums
    return np.ascontiguousarray(hist.T), res


def kernel(**inputs) -> np.ndarray:
    out, _ = run(inputs["inputs"],
                 trace=bool(os.environ.get("KERNEL_TRACE")))
    return out
